# revision 21
# baseline (speedup 1.0000x reference)
"""Trainium2 Bass kernel for nn_ADDMeanM_16595753632500.

out[b] = expm(D_b logm(X_b) D_b), X_b = f[b,0] (64x64 SPD), D_b = diag(w[b]),
B = 8192, data-parallel across 8 NeuronCores (1024 samples each).

Eigh-free fp16 algorithm (batched 64x64 matmuls, fp32 PSUM accumulate):
  tuned coupled Newton-Schulz sqrt chain (2 levels, 4+3 iters) ->
      Y ~ c*X^(1/4), Z ~ c'*X^(-1/4)  (recentered; scale folded into
      final-iteration copy scales alpha/beta)
  S = alpha*Y' - beta*Z' = sinh(T), T = (1/4) log x - log r
  Hp = asinh(S)/4 via 2-term odd series
  A' = w w^T o (Hp + gamma' I); out = expm(4A')^4 (Taylor-4
      Paterson-Stockmeyer + 2 squarings)
Validated offline vs fp64 eigh oracle: max rel err ~1.1e-2 (gate 2e-2).

Layout: 2 samples per 128 partitions (quadrant K=64 matmuls, base
partitions 0/64), GANG=8 pairs side-by-side in the free dim (512-wide
vector ops amortize 16 samples). fp16 everywhere on-chip except PSUM
(fp32) and const blocks; fp16 DMA in AND out (host casts).
"""
import os
import numpy as np

BUFS_WORK = int(os.environ.get("K_BUFS_WORK", "13"))
BUFS_PSA = int(os.environ.get("K_BUFS_PSA", "4"))
BUFS_PSB = int(os.environ.get("K_BUFS_PSB", "2"))
INTERLEAVE = int(os.environ.get("K_INTERLEAVE", "12"))

# ---------------- tuned schedule constants (offline, /root/tune) ----------
LEV0 = [(1.7545051257294326, -0.23803317376081404),
        (1.5353727795763776, -0.3295560584540806),
        (1.5070719222865991, -0.46490504786416914),
        (1.5002355571599766, -0.49882251009023504)]
LEV1 = [(1.6159520526143833, -0.43106748263419),
        (1.504202789356025, -0.47907982016421485),
        (1.5000816689108767, -0.4995916910213691)]
ALPHA = 0.5284185047966153
BETA = 0.47309797345463184
CC = -0.22121679970910058          # log x = 16*Hp + CC
NSQ = 2
GAMMA = CC / (2 ** NSQ)
ASC = [0.25, -0.25 / 6.0, 0.25 * 3.0 / 40.0, -0.25 * 15.0 / 336.0]

N_CORES = 8
B_TOTAL = 8192
SHARD = B_TOTAL // N_CORES
GANG = 8
N = 64
GW = GANG * N                       # 512
NPAIR = SHARD // 2                  # 512
NGANG = NPAIR // GANG               # 64

# const blocks (each GW wide), fp32:
#  0..3 a of LEV0, 4..6 a of LEV1, 7 cf0, 8 gamma/4, 9 one, 10 eight
_CONST_VALS = ([a for (a, b) in LEV0] + [a for (a, b) in LEV1]
               + [ASC[0], GAMMA / 4.0, 1.0, 8.0])
NCONST = len(_CONST_VALS)
# fp16 const blocks (for DVE tensor_tensor adds): 0 one, 1 cf0
_CONST16_VALS = [1.0, ASC[0]]
NCONST16 = len(_CONST16_VALS)


def _host_constants():
    eye = np.eye(N, dtype=np.float32)
    blk = np.zeros((128, NCONST * GW), np.float32)
    for k, v in enumerate(_CONST_VALS):
        for j in range(GANG):
            for t in range(2):
                blk[64 * t:64 * t + 64, k * GW + j * N:k * GW + (j + 1) * N] \
                    = v * eye
    return blk


def _host_constants16():
    eye = np.eye(N, dtype=np.float16)
    blk = np.zeros((128, NCONST16 * GW), np.float16)
    for k, v in enumerate(_CONST16_VALS):
        for j in range(GANG):
            for t in range(2):
                blk[64 * t:64 * t + 64, k * GW + j * N:k * GW + (j + 1) * N] \
                    = np.float16(v) * eye
    return blk


def _rearr(x):
    """[SHARD, 64, 64] -> [NGANG, 128, GW] gang layout (sample s=(g*8+j)*2+t
    lives at partitions 64t..64t+64, cols 64j..64j+64)."""
    v = x.reshape(NGANG, GANG, 2, N, N).transpose(0, 2, 3, 1, 4)
    return np.ascontiguousarray(v.reshape(NGANG, 128, GW))


def _unrearr(y):
    """inverse of _rearr."""
    v = y.reshape(NGANG, 2, N, GANG, N).transpose(0, 3, 1, 2, 4)
    return v.reshape(SHARD, N, N)


def build_nc(ngang=NGANG):
    import concourse.bacc as bacc
    import concourse.mybir as mybir
    import concourse.tile as tile

    f32 = mybir.dt.float32
    f16 = mybir.dt.float16
    nc = bacc.Bacc()
    f_in = nc.declare_dram_parameter("f", [ngang, 128, GW], f16,
                                     isOutput=False)
    wo_in = nc.declare_dram_parameter("wo", [ngang, 128, GW], f16,
                                      isOutput=False)
    cst_in = nc.declare_dram_parameter("cst", [128, NCONST * GW], f32,
                                       isOutput=False)
    cst16_in = nc.declare_dram_parameter("cst16", [128, NCONST16 * GW], f16,
                                         isOutput=False)
    out_d = nc.declare_dram_parameter("out", [ngang, 128, GW], f16,
                                      isOutput=True)

    mult = mybir.AluOpType.mult
    add = mybir.AluOpType.add
    sub = mybir.AluOpType.subtract
    CopyF = mybir.ActivationFunctionType.Copy

    with tile.TileContext(nc) as tc:
        with (
            tc.tile_pool(name="consts", bufs=1) as cpool,
            tc.tile_pool(name="work", bufs=BUFS_WORK) as wpool,
            tc.tile_pool(name="psA", bufs=BUFS_PSA, space="PSUM") as psA,
            tc.tile_pool(name="psB", bufs=BUFS_PSB, space="PSUM") as psB,
        ):
            cst = cpool.tile([128, NCONST * GW], f32)
            nc.sync.dma_start(cst[:], cst_in[:])
            cst16 = cpool.tile([128, NCONST16 * GW], f16)
            nc.sync.dma_start(cst16[:], cst16_in[:])

            def cblk(k):
                return cst[:, k * GW:(k + 1) * GW]

            def cblk16(k):
                return cst16[:, k * GW:(k + 1) * GW]
            cI = cblk(9)
            cI16 = cblk16(0)

            def gang_stages(gi):
                xg = wpool.tile([128, GW], f16, tag="xg")
                wog = wpool.tile([128, GW], f16, tag="wog")
                yz = wpool.tile([128, 2 * GW], f16, tag="yz")
                yz2 = wpool.tile([128, 2 * GW], f16, tag="yz2")
                wt = wpool.tile([128, GW], f16, tag="wt")
                sg = wpool.tile([128, GW], f16, tag="sg")
                ug = wpool.tile([128, GW], f16, tag="ug")
                pg = wpool.tile([128, GW], f16, tag="pg")
                pg2 = wpool.tile([128, GW], f16, tag="pg2")
                t32 = wpool.tile([128, GW], f32, tag="t32")
                ag = xg                      # X dead after L0 i1
                gg = yz[:, 0:GW]             # yz dead after L1 i1
                gg2 = yz[:, GW:2 * GW]
                og = ug                      # U dead after Hp

                nc.sync.dma_start(xg[:], f_in[gi])
                nc.sync.dma_start(wog[:], wo_in[gi])
                yield

                def sl(tile_, j):
                    return tile_[:, j * N:(j + 1) * N]

                def mmq(out_ap, statT_ap, mov_ap):
                    for t in range(2):
                        ps = slice(64 * t, 64 * t + 64)
                        nc.tensor.matmul(out_ap[ps], statT_ap[ps], mov_ap[ps])

                def prod(pool, statT, mov, wide=False, tag="pT"):
                    w = 2 * GW if wide else GW
                    p = pool.tile([128, w], f32, tag=tag)
                    for j in range(GANG):
                        if wide:
                            mmq(p[:, 2 * N * j:2 * N * (j + 1)],
                                sl(statT, j), mov(j))
                        else:
                            mmq(sl(p, j), sl(statT, j), mov(j))
                    return p

                def yzv(tile_):
                    return tile_[:].rearrange("p (j d) -> p j d", d=2 * N)

                def Ysl(tile_, j):
                    return tile_[:, 2 * N * j: 2 * N * j + N]

                def Zsl(tile_, j):
                    return tile_[:, 2 * N * j + N: 2 * N * j + 2 * N]

                def YZsl(tile_, j):
                    return tile_[:, 2 * N * j: 2 * N * (j + 1)]

                # ---- level 0 ----
                # i1: W = a0 I + b0 X -> Z slots; Y1 = W@X
                Yv = yzv(yz)[:, :, 0:N]
                Zv = yzv(yz)[:, :, N:2 * N]
                nc.vector.scalar_tensor_tensor(
                    Zv, xg[:].rearrange("p (j d) -> p j d", d=N),
                    float(LEV0[0][1]),
                    cblk(0)[:].rearrange("p (j d) -> p j d", d=N), mult, add)
                pY = psA.tile([128, GW], f32, tag="pT")
                for j in range(GANG):
                    mmq(sl(pY, j), Zsl(yz, j), sl(xg, j))
                nc.scalar.activation(Yv, pY[:].rearrange("p (j d) -> p j d",
                                                         d=N), CopyF)
                yield

                def ns_iter(src, dst, k, lev, cb, last=False, yonly=False):
                    # T = Z@Y ; W = a I + b T ; [Y'|Z'] = W @ [Y|Z]
                    b = lev[k][1]
                    pT = psA.tile([128, GW], f32, tag="pT")
                    for j in range(GANG):
                        mmq(sl(pT, j), Zsl(src, j), Ysl(src, j))
                    nc.vector.scalar_tensor_tensor(
                        wt[:], pT[:], float(b), cblk(cb), mult, add)
                    if yonly:
                        pYo = psA.tile([128, GW], f32, tag="pT")
                        for j in range(GANG):
                            mmq(sl(pYo, j), sl(wt, j), Ysl(src, j))
                        nc.scalar.activation(yzv(dst)[:, :, 0:N],
                                             pYo[:].rearrange(
                                                 "p (j d) -> p j d", d=N),
                                             CopyF)
                        return
                    pYZ = psB.tile([128, 2 * GW], f32, tag="pYZ")
                    for j in range(GANG):
                        mmq(pYZ[:, 2 * N * j:2 * N * (j + 1)], sl(wt, j),
                            YZsl(src, j))
                    pv = pYZ[:].rearrange("p (j d) -> p j d", d=2 * N)
                    if last:
                        # zb = beta*Z' ; S = alpha*Y' - zb
                        nc.scalar.activation(ug[:].rearrange(
                            "p (j d) -> p j d", d=N), pv[:, :, N:2 * N],
                            CopyF, scale=float(BETA))
                        nc.vector.scalar_tensor_tensor(
                            sg[:].rearrange("p (j d) -> p j d", d=N),
                            pv[:, :, 0:N], float(ALPHA),
                            ug[:].rearrange("p (j d) -> p j d", d=N),
                            mult, sub)
                    else:
                        nc.scalar.activation(dst[:], pYZ[:], CopyF)

                for k in range(1, len(LEV0)):
                    ns_iter(yz, yz, k, LEV0, k)
                    yield

                # ---- level 1 ----
                # i1: W = a I + b Y -> yz2 Z slots ; Ynew = W@Y
                Y2v = yzv(yz2)[:, :, 0:N]
                Z2v = yzv(yz2)[:, :, N:2 * N]
                nc.vector.scalar_tensor_tensor(
                    Z2v, Yv, float(LEV1[0][1]),
                    cblk(4)[:].rearrange("p (j d) -> p j d", d=N), mult, add)
                pY2 = psA.tile([128, GW], f32, tag="pT")
                for j in range(GANG):
                    mmq(sl(pY2, j), Zsl(yz2, j), Ysl(yz, j))
                nc.scalar.activation(Y2v, pY2[:].rearrange(
                    "p (j d) -> p j d", d=N), CopyF)
                yield

                ns_iter(yz2, yz2, 1, LEV1, 5)
                yield
                ns_iter(yz2, yz2, 2, LEV1, 6, last=True)
                yield

                # ---- asinh 3 terms: P = (cf2 U + cf1) U + cf0.
                #      S in block-diag form (yz2 reused, K=128 single-instr
                #      matmuls for U and Hp); U' = cf2*U (ACT copy scale);
                #      P2 = U' + cf1 I (fast TT); P0 = (P2@U')/cf2 + cf0 I
                nc.gpsimd.memset(yz2[:], 0.0)
                v2 = yzv(yz2)
                nc.vector.tensor_copy(
                    v2[0:64, :, 0:N],
                    sg[0:64, :].rearrange("p (j d) -> p j d", d=N))
                nc.scalar.activation(
                    v2[64:128, :, N:2 * N],
                    sg[64:128, :].rearrange("p (j d) -> p j d", d=N), CopyF)
                pU = psA.tile([128, GW], f32, tag="pT")
                for j in range(GANG):
                    nc.tensor.matmul(sl(pU, j), YZsl(yz2, j), sl(sg, j))
                nc.scalar.activation(ug[:], pU[:], CopyF, scale=float(ASC[1]))
                yield
                nc.vector.tensor_tensor(pg[:], ug[:], cblk16(1), add)
                yield
                # Hp = S @ P ; A' = wo o (2 Hp + gamma I)/DEG  (DEG=4 folded)
                pH = psA.tile([128, GW], f32, tag="pT")
                for j in range(GANG):
                    nc.tensor.matmul(sl(pH, j), YZsl(yz2, j), sl(pg, j))
                nc.vector.scalar_tensor_tensor(
                    t32[:], pH[:], float(16.0 / (2 ** NSQ) / 4.0), cblk(8),
                    mult, add)
                nc.gpsimd.tensor_tensor(ag[:], t32[:], wog[:], mult)
                yield

                # ---- exp Taylor-4 PS on A' = A/4:
                #      G = (I + 4A') + (3/32) A2'@(8I + (32/3)A' + A2'),
                #      A2' = (32/3) A'^2
                pA2 = psA.tile([128, GW], f32, tag="pT")
                for j in range(GANG):
                    mmq(sl(pA2, j), sl(ag, j), sl(ag, j))
                nc.scalar.activation(pg[:], pA2[:], CopyF,
                                     scale=float(32.0 / 3.0))
                nc.vector.scalar_tensor_tensor(
                    gg, ag[:], float(32.0 / 3.0), cblk(10), mult, add)
                yield
                nc.vector.tensor_tensor(pg2[:], gg, pg[:], add)
                nc.vector.scalar_tensor_tensor(
                    gg2, ag[:], 4.0, cI, mult, add)
                pG = psA.tile([128, GW], f32, tag="pT")
                for j in range(GANG):
                    mmq(sl(pG, j), sl(pg, j), sl(pg2, j))
                nc.vector.scalar_tensor_tensor(
                    gg, pG[:], float(3.0 / 32.0), gg2, mult, add)

                # ---- 2 squarings ----
                pS1 = psA.tile([128, GW], f32, tag="pT")
                for j in range(GANG):
                    mmq(sl(pS1, j), gg[:, j * N:(j + 1) * N], gg[:, j * N:(j + 1) * N])
                nc.scalar.activation(gg2, pS1[:], CopyF)
                yield
                pS2 = psA.tile([128, GW], f32, tag="pT")
                for j in range(GANG):
                    mmq(sl(pS2, j), gg2[:, j * N:(j + 1) * N], gg2[:, j * N:(j + 1) * N])
                nc.scalar.activation(og[:], pS2[:], CopyF)
                nc.scalar.dma_start(out_d[gi], og[:])

            def run_interleaved(ngang_, width):
                gens = []
                nxt = 0
                while gens or nxt < ngang_:
                    if len(gens) < width and nxt < ngang_:
                        gens.append(gang_stages(nxt))
                        nxt += 1
                    done = []
                    for g in gens:
                        try:
                            next(g)
                        except StopIteration:
                            done.append(g)
                    for g in done:
                        gens.remove(g)

            run_interleaved(ngang, INTERLEAVE)

    nc.compile()
    return nc


_cached = {}


def _get_nc(ngang=NGANG):
    if ngang not in _cached:
        _cached[ngang] = build_nc(ngang)
    return _cached[ngang]


def _in_maps(f, weights):
    f16 = f[:, 0].astype(np.float16)
    w32 = weights.astype(np.float32)
    cst = _host_constants()
    cst16 = _host_constants16()
    in_maps = []
    for c in range(N_CORES):
        sl_ = slice(c * SHARD, (c + 1) * SHARD)
        wc = w32[sl_]
        wo = (wc[:, :, None] * wc[:, None, :]).astype(np.float16)
        in_maps.append({
            "f": _rearr(f16[sl_]),
            "wo": _rearr(wo),
            "cst": cst,
            "cst16": cst16,
        })
    return in_maps


def kernel(f: np.ndarray, weights: np.ndarray) -> np.ndarray:
    from concourse.bass_utils import run_bass_kernel_spmd

    assert f.shape == (B_TOTAL, 1, N, N) and weights.shape == (B_TOTAL, N)
    nc = _get_nc()
    res = run_bass_kernel_spmd(nc, _in_maps(f, weights),
                               core_ids=list(range(N_CORES)))
    out = np.empty((B_TOTAL, 1, N, N), np.float32)
    for c in range(N_CORES):
        out[c * SHARD:(c + 1) * SHARD, 0] = \
            _unrearr(res.results[c]["out"]).astype(np.float32)
    return out


def run_traced(f: np.ndarray, weights: np.ndarray):
    from concourse.bass_utils import run_bass_kernel_spmd

    nc = _get_nc()
    return run_bass_kernel_spmd(nc, _in_maps(f, weights),
                                core_ids=list(range(N_CORES)), trace=True)


# revision 22
# speedup vs baseline: 1.0118x; 1.0118x over previous
"""Trainium2 Bass kernel for nn_ADDMeanM_16595753632500.

out[b] = expm(D_b logm(X_b) D_b), X_b = f[b,0] (64x64 SPD), D_b = diag(w[b]),
B = 8192, data-parallel across 8 NeuronCores (1024 samples each).

Eigh-free fp16 algorithm (batched 64x64 matmuls, fp32 PSUM accumulate):
  tuned coupled Newton-Schulz sqrt chain (2 levels, 4+3 iters) ->
      Y ~ c*X^(1/4), Z ~ c'*X^(-1/4)  (recentered; scale folded into
      final-iteration copy scales alpha/beta)
  S = alpha*Y' - beta*Z' = sinh(T), T = (1/4) log x - log r
  Hp = asinh(S)/4 via 3-term odd Horner series
  A' = w w^T o (Hp + gamma' I); out = expm(4A')^4 (Taylor-4
      Paterson-Stockmeyer + 2 squarings)
Validated offline vs fp64 eigh oracle: max rel err ~5.9e-3 (gate 2e-2).

Layout: 2 samples per 128 partitions (quadrant K=64 matmuls, base
partitions 0/64), GANG=8 pairs side-by-side in the free dim (512-wide
vector ops amortize 16 samples). fp16 everywhere on-chip except PSUM
(fp32) and const blocks; fp16 DMA in AND out (host casts).
"""
import os
import numpy as np

BUFS_WORK = int(os.environ.get("K_BUFS_WORK", "13"))
BUFS_PSA = int(os.environ.get("K_BUFS_PSA", "4"))
BUFS_PSB = int(os.environ.get("K_BUFS_PSB", "2"))
INTERLEAVE = int(os.environ.get("K_INTERLEAVE", "12"))

# ---------------- tuned schedule constants (offline, /root/tune) ----------
LEV0 = [(1.7545051257294326, -0.23803317376081404),
        (1.5353727795763776, -0.3295560584540806),
        (1.5070719222865991, -0.46490504786416914),
        (1.5002355571599766, -0.49882251009023504)]
LEV1 = [(1.6159520526143833, -0.43106748263419),
        (1.504202789356025, -0.47907982016421485),
        (1.5000816689108767, -0.4995916910213691)]
ALPHA = 0.5284185047966153
BETA = 0.47309797345463184
CC = -0.22121679970910058          # log x = 16*Hp + CC
NSQ = 2
GAMMA = CC / (2 ** NSQ)
ASC = [0.25, -0.25 / 6.0, 0.25 * 3.0 / 40.0, -0.25 * 15.0 / 336.0]

N_CORES = 8
B_TOTAL = 8192
SHARD = B_TOTAL // N_CORES
GANG = 8
N = 64
GW = GANG * N                       # 512
NPAIR = SHARD // 2                  # 512
NGANG = NPAIR // GANG               # 64

# const blocks (each GW wide), fp32:
#  0..3 a of LEV0, 4..6 a of LEV1, 7 cf0, 8 gamma/4, 9 one, 10 eight
_CONST_VALS = ([a for (a, b) in LEV0] + [a for (a, b) in LEV1]
               + [ASC[0], GAMMA / 4.0, 1.0, 8.0])
NCONST = len(_CONST_VALS)
# fp16 const blocks (for DVE tensor_tensor adds): 0 one, 1 cf1
_CONST16_VALS = [1.0, ASC[1]]
NCONST16 = len(_CONST16_VALS)


def _host_constants():
    eye = np.eye(N, dtype=np.float32)
    blk = np.zeros((128, NCONST * GW), np.float32)
    for k, v in enumerate(_CONST_VALS):
        for j in range(GANG):
            for t in range(2):
                blk[64 * t:64 * t + 64, k * GW + j * N:k * GW + (j + 1) * N] \
                    = v * eye
    return blk


def _host_constants16():
    eye = np.eye(N, dtype=np.float16)
    blk = np.zeros((128, NCONST16 * GW), np.float16)
    for k, v in enumerate(_CONST16_VALS):
        for j in range(GANG):
            for t in range(2):
                blk[64 * t:64 * t + 64, k * GW + j * N:k * GW + (j + 1) * N] \
                    = np.float16(v) * eye
    return blk


def _rearr(x):
    """[SHARD, 64, 64] -> [NGANG, 128, GW] gang layout (sample s=(g*8+j)*2+t
    lives at partitions 64t..64t+64, cols 64j..64j+64)."""
    v = x.reshape(NGANG, GANG, 2, N, N).transpose(0, 2, 3, 1, 4)
    return np.ascontiguousarray(v.reshape(NGANG, 128, GW))


def _unrearr(y):
    """inverse of _rearr."""
    v = y.reshape(NGANG, 2, N, GANG, N).transpose(0, 3, 1, 2, 4)
    return v.reshape(SHARD, N, N)


def build_nc(ngang=NGANG):
    import concourse.bacc as bacc
    import concourse.mybir as mybir
    import concourse.tile as tile

    f32 = mybir.dt.float32
    f16 = mybir.dt.float16
    nc = bacc.Bacc()
    f_in = nc.declare_dram_parameter("f", [ngang, 128, GW], f16,
                                     isOutput=False)
    wo_in = nc.declare_dram_parameter("wo", [ngang, 128, GW], f16,
                                      isOutput=False)
    cst_in = nc.declare_dram_parameter("cst", [128, NCONST * GW], f32,
                                       isOutput=False)
    cst16_in = nc.declare_dram_parameter("cst16", [128, NCONST16 * GW], f16,
                                         isOutput=False)
    out_d = nc.declare_dram_parameter("out", [ngang, 128, GW], f16,
                                      isOutput=True)

    mult = mybir.AluOpType.mult
    add = mybir.AluOpType.add
    sub = mybir.AluOpType.subtract
    CopyF = mybir.ActivationFunctionType.Copy

    with tile.TileContext(nc) as tc:
        with (
            tc.tile_pool(name="consts", bufs=1) as cpool,
            tc.tile_pool(name="work", bufs=BUFS_WORK) as wpool,
            tc.tile_pool(name="psA", bufs=BUFS_PSA, space="PSUM") as psA,
            tc.tile_pool(name="psB", bufs=BUFS_PSB, space="PSUM") as psB,
        ):
            cst = cpool.tile([128, NCONST * GW], f32)
            nc.sync.dma_start(cst[:], cst_in[:])
            cst16 = cpool.tile([128, NCONST16 * GW], f16)
            nc.sync.dma_start(cst16[:], cst16_in[:])

            def cblk(k):
                return cst[:, k * GW:(k + 1) * GW]

            def cblk16(k):
                return cst16[:, k * GW:(k + 1) * GW]
            cI = cblk(9)
            cI16 = cblk16(0)

            def gang_stages(gi):
                xg = wpool.tile([128, GW], f16, tag="xg")
                wog = wpool.tile([128, GW], f16, tag="wog")
                yz = wpool.tile([128, 2 * GW], f16, tag="yz")
                yz2 = wpool.tile([128, 2 * GW], f16, tag="yz2")
                wt = wpool.tile([128, GW], f16, tag="wt")
                sg = wpool.tile([128, GW], f16, tag="sg")
                ug = wpool.tile([128, GW], f16, tag="ug")
                pg = wpool.tile([128, GW], f16, tag="pg")
                pg2 = wpool.tile([128, GW], f16, tag="pg2")
                t32 = wpool.tile([128, GW], f32, tag="t32")
                ag = xg                      # X dead after L0 i1
                gg = yz[:, 0:GW]             # yz dead after L1 i1
                gg2 = yz[:, GW:2 * GW]
                og = ug                      # U dead after Hp

                nc.sync.dma_start(xg[:], f_in[gi])
                nc.sync.dma_start(wog[:], wo_in[gi])
                yield

                def sl(tile_, j):
                    return tile_[:, j * N:(j + 1) * N]

                def mmq(out_ap, statT_ap, mov_ap):
                    for t in range(2):
                        ps = slice(64 * t, 64 * t + 64)
                        nc.tensor.matmul(out_ap[ps], statT_ap[ps], mov_ap[ps])

                def prod(pool, statT, mov, wide=False, tag="pT"):
                    w = 2 * GW if wide else GW
                    p = pool.tile([128, w], f32, tag=tag)
                    for j in range(GANG):
                        if wide:
                            mmq(p[:, 2 * N * j:2 * N * (j + 1)],
                                sl(statT, j), mov(j))
                        else:
                            mmq(sl(p, j), sl(statT, j), mov(j))
                    return p

                def yzv(tile_):
                    return tile_[:].rearrange("p (j d) -> p j d", d=2 * N)

                def Ysl(tile_, j):
                    return tile_[:, 2 * N * j: 2 * N * j + N]

                def Zsl(tile_, j):
                    return tile_[:, 2 * N * j + N: 2 * N * j + 2 * N]

                def YZsl(tile_, j):
                    return tile_[:, 2 * N * j: 2 * N * (j + 1)]

                # ---- level 0 ----
                # i1: W = a0 I + b0 X -> Z slots; Y1 = W@X
                Yv = yzv(yz)[:, :, 0:N]
                Zv = yzv(yz)[:, :, N:2 * N]
                nc.vector.scalar_tensor_tensor(
                    Zv, xg[:].rearrange("p (j d) -> p j d", d=N),
                    float(LEV0[0][1]),
                    cblk(0)[:].rearrange("p (j d) -> p j d", d=N), mult, add)
                pY = psA.tile([128, GW], f32, tag="pT")
                for j in range(GANG):
                    mmq(sl(pY, j), Zsl(yz, j), sl(xg, j))
                nc.scalar.activation(Yv, pY[:].rearrange("p (j d) -> p j d",
                                                         d=N), CopyF)
                yield

                def ns_iter(src, dst, k, lev, cb, last=False, yonly=False):
                    # T = Z@Y ; W = a I + b T ; [Y'|Z'] = W @ [Y|Z]
                    b = lev[k][1]
                    pT = psA.tile([128, GW], f32, tag="pT")
                    for j in range(GANG):
                        mmq(sl(pT, j), Zsl(src, j), Ysl(src, j))
                    nc.vector.scalar_tensor_tensor(
                        wt[:], pT[:], float(b), cblk(cb), mult, add)
                    if yonly:
                        pYo = psA.tile([128, GW], f32, tag="pT")
                        for j in range(GANG):
                            mmq(sl(pYo, j), sl(wt, j), Ysl(src, j))
                        nc.scalar.activation(yzv(dst)[:, :, 0:N],
                                             pYo[:].rearrange(
                                                 "p (j d) -> p j d", d=N),
                                             CopyF)
                        return
                    pYZ = psB.tile([128, 2 * GW], f32, tag="pYZ")
                    for j in range(GANG):
                        mmq(pYZ[:, 2 * N * j:2 * N * (j + 1)], sl(wt, j),
                            YZsl(src, j))
                    pv = pYZ[:].rearrange("p (j d) -> p j d", d=2 * N)
                    if last:
                        # zb = beta*Z' ; S = alpha*Y' - zb
                        nc.scalar.activation(ug[:].rearrange(
                            "p (j d) -> p j d", d=N), pv[:, :, N:2 * N],
                            CopyF, scale=float(BETA))
                        nc.vector.scalar_tensor_tensor(
                            sg[:].rearrange("p (j d) -> p j d", d=N),
                            pv[:, :, 0:N], float(ALPHA),
                            ug[:].rearrange("p (j d) -> p j d", d=N),
                            mult, sub)
                    else:
                        nc.scalar.activation(dst[:], pYZ[:], CopyF)

                for k in range(1, len(LEV0)):
                    ns_iter(yz, yz, k, LEV0, k)
                    yield

                # ---- level 1 ----
                # i1: W = a I + b Y -> yz2 Z slots ; Ynew = W@Y
                Y2v = yzv(yz2)[:, :, 0:N]
                Z2v = yzv(yz2)[:, :, N:2 * N]
                nc.vector.scalar_tensor_tensor(
                    Z2v, Yv, float(LEV1[0][1]),
                    cblk(4)[:].rearrange("p (j d) -> p j d", d=N), mult, add)
                pY2 = psA.tile([128, GW], f32, tag="pT")
                for j in range(GANG):
                    mmq(sl(pY2, j), Zsl(yz2, j), Ysl(yz, j))
                nc.scalar.activation(Y2v, pY2[:].rearrange(
                    "p (j d) -> p j d", d=N), CopyF)
                yield

                ns_iter(yz2, yz2, 1, LEV1, 5)
                yield
                ns_iter(yz2, yz2, 2, LEV1, 6, last=True)
                yield

                # ---- asinh 3 terms: P = (cf2 U + cf1) U + cf0.
                #      S in block-diag form (yz2 reused, K=128 single-instr
                #      matmuls for U and Hp); U' = cf2*U (ACT copy scale);
                #      P2 = U' + cf1 I (fast TT); P0 = (P2@U')/cf2 + cf0 I
                nc.gpsimd.memset(yz2[:], 0.0)
                v2 = yzv(yz2)
                nc.vector.tensor_copy(
                    v2[0:64, :, 0:N],
                    sg[0:64, :].rearrange("p (j d) -> p j d", d=N))
                nc.scalar.activation(
                    v2[64:128, :, N:2 * N],
                    sg[64:128, :].rearrange("p (j d) -> p j d", d=N), CopyF)
                pU = psA.tile([128, GW], f32, tag="pT")
                for j in range(GANG):
                    nc.tensor.matmul(sl(pU, j), YZsl(yz2, j), sl(sg, j))
                nc.scalar.activation(ug[:], pU[:], CopyF, scale=float(ASC[2]))
                yield
                nc.vector.tensor_tensor(pg[:], ug[:], cblk16(1), add)
                pP = psA.tile([128, GW], f32, tag="pT")
                for j in range(GANG):
                    mmq(sl(pP, j), sl(ug, j), sl(pg, j))
                nc.vector.scalar_tensor_tensor(
                    pg2[:], pP[:], float(1.0 / ASC[2]), cblk(7), mult, add)
                yield
                # Hp = S @ P0 ; A' = wo o (2 Hp + gamma I)/DEG  (DEG=4 folded)
                pH = psA.tile([128, GW], f32, tag="pT")
                for j in range(GANG):
                    nc.tensor.matmul(sl(pH, j), YZsl(yz2, j), sl(pg2, j))
                nc.vector.scalar_tensor_tensor(
                    t32[:], pH[:], float(16.0 / (2 ** NSQ) / 4.0), cblk(8),
                    mult, add)
                nc.gpsimd.tensor_tensor(ag[:], t32[:], wog[:], mult)
                yield

                # ---- exp Taylor-4 PS on A' = A/4:
                #      G = (I + 4A') + (3/32) A2'@(8I + (32/3)A' + A2'),
                #      A2' = (32/3) A'^2
                pA2 = psA.tile([128, GW], f32, tag="pT")
                for j in range(GANG):
                    mmq(sl(pA2, j), sl(ag, j), sl(ag, j))
                nc.scalar.activation(pg[:], pA2[:], CopyF,
                                     scale=float(32.0 / 3.0))
                nc.vector.scalar_tensor_tensor(
                    gg, ag[:], float(32.0 / 3.0), cblk(10), mult, add)
                yield
                nc.vector.tensor_tensor(pg2[:], gg, pg[:], add)
                nc.vector.scalar_tensor_tensor(
                    gg2, ag[:], 4.0, cI, mult, add)
                pG = psA.tile([128, GW], f32, tag="pT")
                for j in range(GANG):
                    mmq(sl(pG, j), sl(pg, j), sl(pg2, j))
                nc.vector.scalar_tensor_tensor(
                    gg, pG[:], float(3.0 / 32.0), gg2, mult, add)

                # ---- 2 squarings ----
                pS1 = psA.tile([128, GW], f32, tag="pT")
                for j in range(GANG):
                    mmq(sl(pS1, j), gg[:, j * N:(j + 1) * N], gg[:, j * N:(j + 1) * N])
                nc.scalar.activation(gg2, pS1[:], CopyF)
                yield
                pS2 = psA.tile([128, GW], f32, tag="pT")
                for j in range(GANG):
                    mmq(sl(pS2, j), gg2[:, j * N:(j + 1) * N], gg2[:, j * N:(j + 1) * N])
                nc.scalar.activation(og[:], pS2[:], CopyF)
                nc.scalar.dma_start(out_d[gi], og[:])

            def run_interleaved(ngang_, width):
                gens = []
                nxt = 0
                while gens or nxt < ngang_:
                    if len(gens) < width and nxt < ngang_:
                        gens.append(gang_stages(nxt))
                        nxt += 1
                    done = []
                    for g in gens:
                        try:
                            next(g)
                        except StopIteration:
                            done.append(g)
                    for g in done:
                        gens.remove(g)

            run_interleaved(ngang, INTERLEAVE)

    nc.compile()
    return nc


_cached = {}


def _get_nc(ngang=NGANG):
    if ngang not in _cached:
        _cached[ngang] = build_nc(ngang)
    return _cached[ngang]


def _in_maps(f, weights):
    f16 = f[:, 0].astype(np.float16)
    w32 = weights.astype(np.float32)
    cst = _host_constants()
    cst16 = _host_constants16()
    in_maps = []
    for c in range(N_CORES):
        sl_ = slice(c * SHARD, (c + 1) * SHARD)
        wc = w32[sl_]
        wo = (wc[:, :, None] * wc[:, None, :]).astype(np.float16)
        in_maps.append({
            "f": _rearr(f16[sl_]),
            "wo": _rearr(wo),
            "cst": cst,
            "cst16": cst16,
        })
    return in_maps


def kernel(f: np.ndarray, weights: np.ndarray) -> np.ndarray:
    from concourse.bass_utils import run_bass_kernel_spmd

    assert f.shape == (B_TOTAL, 1, N, N) and weights.shape == (B_TOTAL, N)
    nc = _get_nc()
    res = run_bass_kernel_spmd(nc, _in_maps(f, weights),
                               core_ids=list(range(N_CORES)))
    out = np.empty((B_TOTAL, 1, N, N), np.float32)
    for c in range(N_CORES):
        out[c * SHARD:(c + 1) * SHARD, 0] = \
            _unrearr(res.results[c]["out"]).astype(np.float32)
    return out


def run_traced(f: np.ndarray, weights: np.ndarray):
    from concourse.bass_utils import run_bass_kernel_spmd

    nc = _get_nc()
    return run_bass_kernel_spmd(nc, _in_maps(f, weights),
                                core_ids=list(range(N_CORES)), trace=True)
